# revision 37
# baseline (speedup 1.0000x reference)
"""Multi-head attention (nn_Attention) for 8 Trainium2 NeuronCores.

Sharding: tensor-parallel over heads (2 heads per core). Each core computes
qkv projection for its head slice from the full input, full attention for its
2 heads, and a partial output projection; partials are summed on the host.

Layout strategy (per core):
  - qkv^T = W_slice @ x^T computed with contraction (c=1024) on the partition
    dim; produces q^T/k^T [128=2*64 head dims, tokens] directly in the
    orientation the S^T matmuls need.
  - S^T tiles [128 keys, 512 queries x 2 heads] via row-tiled matmul pairs
    (head A on array rows 0:63, head B on 64:127) which execute CONCURRENTLY
    on the PE's row groups.
  - softmax without max-subtraction (|S| < 7 for these inputs): exp on ACT
    (PSUM -> SBUF bf16), then O^T = (E^T [v|ones]) with the ones columns
    producing the softmax normalizer Z on the opposite 64 partitions.
  - Z rows are moved onto the O rows' partitions with a swap-halves
    permutation matmul, reciprocal via the fast custom DVE op, and the
    normalization fused into the PSUM->SBUF copy (tensor_mul). The chain is
    deferred into the next qc's first slot so it never blocks the S/exp
    pipeline at qc boundaries.
  - proj: out_partial[tokens, feat] = O^T_cat.T @ w_projT_slice, summed on
    host across cores. v-bias is folded into b_proj on the host (softmax
    weights sum to 1, so the v bias adds a constant to O).
All matmul operands are bf16 (fp32 streams at ~2 cycles/row on HW, bf16 at
1); intermediates accumulate in fp32 PSUM. Output partials ship as bf16.
"""

import os
import numpy as np

N_CORES = 8
DIM = 1024
N_HEADS = 16
HEAD_DIM = 64
SCALE = HEAD_DIM ** -0.5
B, N = 4, 2048
TOK = B * N  # 8192
NB_C = DIM // 128   # 8 contraction tiles for qkv
NB_J = N // 128     # 16 key tiles per batch
NB_QC = N // 512    # 4 query chunks per batch
NB_TCH = N // 512   # 4 token chunks per batch (qkv)

_cache = {}


def _build():
    if "nc" in _cache:
        return _cache["nc"]
    import concourse.bacc as bacc
    import concourse.mybir as mybir
    from concourse.tile import TileContext

    f32 = mybir.dt.float32
    bf16 = mybir.dt.bfloat16
    Exp = mybir.ActivationFunctionType.Exp

    nc = bacc.Bacc(None, target_bir_lowering=False)
    xT_d = nc.dram_tensor("xT", [DIM, TOK], bf16, kind="ExternalInput")
    wqkvT_d = nc.dram_tensor("wqkvT", [DIM, 384], bf16, kind="ExternalInput")
    bias_d = nc.dram_tensor("bias", [128, 3], f32, kind="ExternalInput")
    wprojT_d = nc.dram_tensor("wprojT", [128, DIM], bf16, kind="ExternalInput")
    ident_d = nc.dram_tensor("ident", [128, 128], bf16, kind="ExternalInput")
    swap_d = nc.dram_tensor("swap", [128, 128], bf16, kind="ExternalInput")
    out_d = nc.dram_tensor("out", [TOK, DIM], bf16, kind="ExternalOutput")

    with TileContext(nc) as tc:
        with tc.tile_pool(name="sbuf", bufs=1) as sb, \
             tc.tile_pool(name="psum", bufs=1, space="PSUM") as ps:
            # constants / weights
            # weights/constants go out on the ACT engine's DMA queue so their
            # issue overlaps the x staging on the sync queue (ACT is idle in
            # the prologue).
            wqkv_t = sb.tile([128, NB_C, 384], bf16, tag="wqkv")
            _wsrc = wqkvT_d[:, :].rearrange("(ct p) r -> p ct r", p=128)
            for ct in range(0, NB_C, 2):
                nc.scalar.dma_start(wqkv_t[:, ct:ct + 2, :],
                                    _wsrc[:, ct:ct + 2, :])
            wproj_t = sb.tile([128, DIM], bf16, tag="wproj")
            nc.scalar.dma_start(wproj_t, wprojT_d[:, :])
            bias_t = sb.tile([128, 3], f32, tag="bias")
            nc.scalar.dma_start(bias_t, bias_d[:, :])
            ident_f = sb.tile([128, 128], bf16, tag="ident")
            nc.scalar.dma_start(ident_f, ident_d[:, :])
            swap_t = sb.tile([128, 128], bf16, tag="swap")
            nc.scalar.dma_start(swap_t, swap_d[:, :])
            ones_t = sb.tile([128, 1], bf16, tag="ones")
            nc.vector.memset(ones_t, 1.0)
            # preload the exp table set during the DMA lead-in
            warm_t = sb.tile([128, 1], f32, tag="warm")
            nc.scalar.activation(warm_t, ones_t, Exp)

            def alloc_batch_tiles():
                qT_t = sb.tile([128, N], bf16, tag="qT", bufs=2)
                kT_t = sb.tile([128, N], bf16, tag="kT", bufs=2)
                # v laid out [tok128, head, ktile, 128] with ones columns:
                # head A block cols = [v_A(64) | ones(64)], head B = [ones | v_B]
                v_t = sb.tile([128, 2, NB_J, 128], bf16, tag="v", bufs=2)
                nc.vector.tensor_copy(
                    v_t[:, 0, :, 64:128],
                    ones_t[:, None, :].broadcast_to([128, NB_J, 64]))
                nc.vector.tensor_copy(
                    v_t[:, 1, :, 0:64],
                    ones_t[:, None, :].broadcast_to([128, NB_J, 64]))
                return qT_t, kT_t, v_t

            def dma_xstage(b, tch, split=1):
                xst = sb.tile([128, NB_C, 512], bf16, tag="xst", bufs=4)
                t0 = b * N + tch * 512
                src = (xT_d[:, t0:t0 + 512]
                       .rearrange("(ct p) t -> p ct t", p=128))
                step = NB_C // split
                for c0 in range(0, NB_C, step):
                    nc.sync.dma_start(xst[:, c0:c0 + step, :],
                                      src[:, c0:c0 + step, :])
                return xst

            def qkv_items(tiles, tch, xst):
                # q/k/v projections for one 512-token chunk, sliced into
                # ~2-matmul filler items so no single slot of the attention
                # loop carries a bulky block that would delay the next S pair
                # (and thereby starve the exp engine).
                qT_t, kT_t, v_t = tiles
                state = {}
                items = []

                def mk(r, c0):
                    def f():
                        if c0 == 0:
                            state[r] = ps.tile([128, 512], f32, name="qp",
                                               tag="misc", bufs=2)
                        qp = state[r]
                        for ct in (c0, c0 + 1):
                            nc.tensor.matmul(
                                qp, wqkv_t[:, ct, r * 128:(r + 1) * 128],
                                xst[:, ct, :],
                                start=(ct == 0), stop=(ct == NB_C - 1))
                        if c0 == NB_C - 2:
                            if r == 0:
                                nc.vector.tensor_scalar_add(
                                    qT_t[:, tch * 512:(tch + 1) * 512], qp,
                                    bias_t[:, 0:1])
                            elif r == 1:
                                nc.vector.tensor_scalar_add(
                                    kT_t[:, tch * 512:(tch + 1) * 512], qp,
                                    bias_t[:, 1:2])
                            else:
                                vst = sb.tile([128, 512], bf16, name="vst",
                                              tag="vtst", bufs=2)
                                nc.vector.tensor_copy(vst, qp)
                                state["vst"] = vst
                    return f

                def mk_tr(s0):
                    def f():
                        vst = state["vst"]
                        for s in (s0, s0 + 1):
                            trp = ps.tile([128, 128], bf16, name="trp",
                                          tag="misc", bufs=2)
                            nc.tensor.transpose(
                                trp, vst[:, s * 128:(s + 1) * 128], ident_f)
                            j = tch * 4 + s
                            nc.vector.tensor_copy(v_t[:, 0, j, 0:64],
                                                  trp[:, 0:64])
                            nc.vector.tensor_copy(v_t[:, 1, j, 64:128],
                                                  trp[:, 64:128])
                    return f

                for r in (0, 1, 2):
                    for c0 in range(0, NB_C, 2):
                        items.append(mk(r, c0))
                items += [mk_tr(0), mk_tr(2)]
                return items

            def proj_mm(prev, idx):
                # one (ts, fc) output tile of the deferred projection
                ot_p, b_p, qc_p = prev
                ts, fc = divmod(idx, 2)
                pj = ps.tile([128, 512], f32, tag="misc", bufs=2)
                nc.tensor.matmul(
                    pj, ot_p[:, ts * 128:(ts + 1) * 128],
                    wproj_t[:, fc * 512:(fc + 1) * 512], start=True, stop=True)
                ost = sb.tile([128, 512], bf16, tag="ost", bufs=6)
                nc.vector.tensor_copy(ost, pj)
                row0 = b_p * N + qc_p * 512 + ts * 128
                nc.sync.dma_start(
                    out_d[row0:row0 + 128, fc * 512:(fc + 1) * 512], ost)

            def finish_norm(pending):
                # swap Z rows onto O rows' lanes via a permutation matmul,
                # then reciprocal + normalization fused into the PSUM drain.
                z_p, oA_p, oB_p, b_p, qc_p = pending
                zsw = ps.tile([128, 512], f32, tag="misc", bufs=2)
                nc.tensor.matmul(zsw, swap_t, z_p, start=True, stop=True)
                r_t = sb.tile([128, 512], f32, tag="rt", bufs=2)
                nc.vector.reciprocal_approx_fast(out=r_t, in_=zsw)
                ot = sb.tile([128, 512], bf16, tag="ot", bufs=2)
                nc.vector.tensor_mul(ot[0:64, :], oA_p[0:64, :], r_t[0:64, :])
                nc.vector.tensor_mul(ot[64:128, :], oB_p[64:128, :],
                                     r_t[64:128, :])
                return (ot, b_p, qc_p)

            # ---- prologue: full QKV for batch 0 ----
            tiles = alloc_batch_tiles()
            xsts = [dma_xstage(0, t, split=s)
                    for t, s in zip(range(NB_TCH), (8, 8, 4, 4))]
            for tch in range(NB_TCH):
                for it in qkv_items(tiles, tch, xsts[tch]):
                    it()

            prev = None      # deferred projection: (ot, b, qc)
            pending = None   # deferred normalization: (z, oA, oB, b, qc)
            xst_q = dma_xstage(1, 0) if B > 1 else None
            for b in range(B):
                qT_t, kT_t, v_t = tiles
                if b + 1 < B:
                    next_tiles = alloc_batch_tiles()
                for qc in range(NB_QC):
                    # x chunk for this qc's qkv filler was prefetched one qc
                    # ago; issue the following one now.
                    xst_cur = xst_q
                    nb, nqc = (b, qc + 1) if qc + 1 < NB_QC else (b + 1, 0)
                    xst_q = (dma_xstage(nb + 1, nqc)
                             if nb + 1 < B else None)
                    fillers = (qkv_items(next_tiles, qc, xst_cur)
                               if b + 1 < B else [])
                    q_sl = slice(qc * 512, (qc + 1) * 512)
                    oA = oB = None  # allocated lazily at first PV use: the
                    # pool's WAR-wait (vs the previous qc's z-copies) must not
                    # sit ahead of this qc's S matmuls in the PE queue.
                    # j-loop at key-tile-PAIR granularity: grouping the
                    # 64-row QK MMs and 128-row PV MMs reduces row-shape
                    # transitions so LDWEIGHTS hides under matmuls. Previous
                    # qc's proj and next batch's qkv chunks fill PE while ACT
                    # runs exp. The S schedule is staggered (1 tile in the
                    # first and last slots, 2 in between) so the first S of a
                    # qc only needs the st slot of j14, which ACT has already
                    # drained -- no boundary wait on the trailing exp.
                    s_at = {0: (0,), 8: (15,)}
                    for mm_ in range(1, 8):
                        s_at[mm_] = (2 * mm_ - 1, 2 * mm_)
                    pv_at = {9: (14, 15)}
                    for mm_ in range(2, 9):
                        pv_at[mm_] = (2 * mm_ - 4, 2 * mm_ - 3)
                    e_pend = [None] * NB_J
                    for m in range(10):
                        for j in s_at.get(m, ()):
                            k_sl = slice(j * 128, (j + 1) * 128)
                            st = ps.tile([128, 1024], f32, tag="st", bufs=2)
                            nc.tensor.matmul(
                                st[:, 0:512], kT_t[0:64, k_sl],
                                qT_t[0:64, q_sl], start=True, stop=True)
                            nc.tensor.matmul(
                                st[:, 512:1024], kT_t[64:128, k_sl],
                                qT_t[64:128, q_sl], start=True, stop=True,
                                tile_position=(64, 0))
                            e_t = sb.tile([128, 1024], bf16, tag="e", bufs=8)
                            nc.scalar.activation(e_t, st, Exp)
                            e_pend[j] = e_t
                        if m == 0 and pending is not None:
                            prev = finish_norm(pending)
                            pending = None
                        if prev is not None and 1 <= m <= 8:
                            proj_mm(prev, m - 1)
                        take = 2 if m < 4 else 1
                        while take and fillers:
                            fillers.pop(0)()
                            take -= 1
                        for j in pv_at.get(m, ()):
                            if oA is None:
                                oA = ps.tile([128, 512], f32, tag="oA",
                                             bufs=1)
                                oB = ps.tile([128, 512], f32, tag="oB",
                                             bufs=1)
                            e_p = e_pend[j]
                            nc.tensor.matmul(
                                oA, v_t[:, 0, j, :], e_p[:, 0:512],
                                start=(j == 0), stop=(j == NB_J - 1))
                            nc.tensor.matmul(
                                oB, v_t[:, 1, j, :], e_p[:, 512:1024],
                                start=(j == 0), stop=(j == NB_J - 1))
                    # stage Z into SBUF now; the two halves go out on
                    # different engines (ACT has drained its queue by the qc
                    # boundary) so the chain clears the PE's WAR-wait sooner.
                    z_st = sb.tile([128, 512], bf16, tag="zst", bufs=2)
                    nc.scalar.copy(z_st[64:128, :], oA[64:128, :])
                    nc.vector.tensor_copy(z_st[0:64, :], oB[0:64, :])
                    pending = (z_st, oA, oB, b, qc)
                if b + 1 < B:
                    tiles = next_tiles
            # tail: last qc's normalization + projection
            prev = finish_norm(pending)
            for idx in range(8):
                proj_mm(prev, idx)

    nc.compile()
    _cache["nc"] = nc
    return nc


def _ensure_ntff_hook():
    """Register the axon NTFF profile hook (antenv.axon_hooks) if absent.

    The agent image's antenv stub lacks axon_hooks, so trn_boot's hook
    registration silently degrades; recreate it here via the same ctypes
    recipe so run_bass_kernel_spmd(trace=True) can capture HW profiles.
    """
    import sys
    import types
    import ctypes
    import contextlib

    try:
        from antenv.axon_hooks import get_axon_ntff_profile_hook
        if get_axon_ntff_profile_hook() is not None:
            return
    except ImportError:
        mod = types.ModuleType("antenv.axon_hooks")
        mod._hook = None
        mod.get_axon_ntff_profile_hook = lambda: mod._hook

        def _set(h):
            mod._hook = h
        mod.set_axon_ntff_profile_hook = _set
        sys.modules["antenv.axon_hooks"] = mod
        import antenv
        antenv.axon_hooks = mod

    so_path = "/opt/axon/libaxon_pjrt.so"
    if not os.path.exists(so_path):
        return
    lib = ctypes.CDLL(so_path)
    if not hasattr(lib, "axon_start_nrt_profile"):
        return
    lib.axon_start_nrt_profile.argtypes = [
        ctypes.POINTER(ctypes.c_int64), ctypes.c_size_t]
    lib.axon_start_nrt_profile.restype = ctypes.c_int64
    lib.axon_stop_nrt_profile.argtypes = [ctypes.c_char_p]
    lib.axon_stop_nrt_profile.restype = ctypes.c_int64

    @contextlib.contextmanager
    def _hook(output_dir, device_ids):
        # the .so's GLOBAL_CLIENT is only set once something executes
        import jax
        jax.block_until_ready(
            jax.jit(lambda a: a + 1)(jax.numpy.zeros((8,), jax.numpy.float32)))
        if device_ids:
            ids = (ctypes.c_int64 * len(device_ids))(*device_ids)
            rc = lib.axon_start_nrt_profile(ids, len(device_ids))
        else:
            rc = lib.axon_start_nrt_profile(None, 0)
        if rc != 0:
            raise RuntimeError(f"axon_start_nrt_profile rc={rc}")
        try:
            yield
        finally:
            n = lib.axon_stop_nrt_profile(str(output_dir).encode())
            print(f"profile: {n} file(s) written to {output_dir}")

    from antenv.axon_hooks import set_axon_ntff_profile_hook
    set_axon_ntff_profile_hook(_hook)


def kernel(x, w_qkv, b_qkv, w_proj, b_proj):
    import ml_dtypes
    from concourse.bass_utils import run_bass_kernel_spmd

    bf16 = ml_dtypes.bfloat16
    nc = _build()
    x = np.asarray(x, dtype=np.float32)
    w_qkv = np.asarray(w_qkv, dtype=np.float32)
    b_qkv = np.asarray(b_qkv, dtype=np.float32)
    w_proj = np.asarray(w_proj, dtype=np.float32)
    b_proj = np.asarray(b_proj, dtype=np.float32)

    xT = np.ascontiguousarray(x.reshape(TOK, DIM).T).astype(bf16)
    ident = np.eye(128, dtype=np.float32).astype(bf16)
    # permutation that swaps partition halves: out = swap.T @ z
    swap = np.zeros((128, 128), dtype=np.float32)
    swap[np.arange(64) + 64, np.arange(64)] = 1.0
    swap[np.arange(64), np.arange(64) + 64] = 1.0
    swap = swap.astype(bf16)

    in_maps = []
    for c in range(N_CORES):
        sl = slice(HEAD_DIM * 2 * c, HEAD_DIM * 2 * c + 128)
        wq = w_qkv[0 * DIM:1 * DIM][sl] * SCALE
        wk = w_qkv[1 * DIM:2 * DIM][sl]
        wv = w_qkv[2 * DIM:3 * DIM][sl]
        wqkvT = np.ascontiguousarray(
            np.concatenate([wq, wk, wv], 0).T).astype(bf16)
        bq = b_qkv[0 * DIM:1 * DIM][sl] * SCALE
        bk = b_qkv[1 * DIM:2 * DIM][sl]
        bias = np.ascontiguousarray(
            np.stack([bq, bk, np.zeros_like(bq)], 1))
        wprojT = np.ascontiguousarray(w_proj[:, sl].T).astype(bf16)
        in_maps.append({"xT": xT, "wqkvT": wqkvT, "bias": bias,
                        "wprojT": wprojT, "ident": ident, "swap": swap})

    trace = os.environ.get("BASS_KERNEL_TRACE", "0") == "1"
    if trace:
        _ensure_ntff_hook()
    res = run_bass_kernel_spmd(nc, in_maps, list(range(N_CORES)), trace=trace)
    if trace:
        _cache["last_exec_time_ns"] = res.exec_time_ns
        _cache["last_mean_exec_time_ns"] = res.mean_exec_time_ns

    out = np.zeros((TOK, DIM), dtype=np.float64)
    for c in range(N_CORES):
        out += res.results[c]["out"].astype(np.float64)
    # v-bias contributes a constant (softmax weights sum to 1): fold into
    # the projection bias here instead of adding it on-device.
    out += b_proj + b_qkv[2 * DIM:3 * DIM] @ w_proj.T
    return out.reshape(B, N, DIM).astype(np.float32)


# revision 40
# speedup vs baseline: 1.1895x; 1.1895x over previous
"""Multi-head attention (nn_Attention) for 8 Trainium2 NeuronCores.

Sharding: tensor-parallel over heads (2 heads per core). Each core computes
qkv projection for its head slice from the full input, full attention for its
2 heads, and a partial output projection; partials are summed on the host.

Layout strategy (per core):
  - qkv^T = W_slice @ x^T computed with contraction (c=1024) on the partition
    dim; produces q^T/k^T [128=2*64 head dims, tokens] directly in the
    orientation the S^T matmuls need.
  - S^T tiles [128 keys, 512 queries x 2 heads] via row-tiled matmul pairs
    (head A on array rows 0:63, head B on 64:127) which execute CONCURRENTLY
    on the PE's row groups.
  - softmax without max-subtraction (|S| < 7 for these inputs): exp on ACT
    (PSUM -> SBUF bf16), then O^T = (E^T [v|ones]) with the ones columns
    producing the softmax normalizer Z on the opposite 64 partitions.
  - Z rows are moved onto the O rows' partitions with a swap-halves
    permutation matmul, reciprocal via the fast custom DVE op, and the
    normalization fused into the PSUM->SBUF copy (tensor_mul). The chain is
    deferred into the next qc's first slot so it never blocks the S/exp
    pipeline at qc boundaries.
  - proj: out_partial[tokens, feat] = O^T_cat.T @ w_projT_slice, summed on
    host across cores. v-bias is folded into b_proj on the host (softmax
    weights sum to 1, so the v bias adds a constant to O).
All matmul operands are bf16 (fp32 streams at ~2 cycles/row on HW, bf16 at
1); intermediates accumulate in fp32 PSUM. Output partials ship as bf16.
"""

import os
import numpy as np

N_CORES = 8
DIM = 1024
N_HEADS = 16
HEAD_DIM = 64
SCALE = HEAD_DIM ** -0.5
B, N = 4, 2048
TOK = B * N  # 8192
NB_C = DIM // 128   # 8 contraction tiles for qkv
NB_J = N // 128     # 16 key tiles per batch
NB_QC = N // 512    # 4 query chunks per batch
NB_TCH = N // 512   # 4 token chunks per batch (qkv)

_cache = {}


def _build():
    if "nc" in _cache:
        return _cache["nc"]
    import concourse.bacc as bacc
    import concourse.mybir as mybir
    from concourse.tile import TileContext

    f32 = mybir.dt.float32
    bf16 = mybir.dt.bfloat16
    Exp = mybir.ActivationFunctionType.Exp

    nc = bacc.Bacc(None, target_bir_lowering=False)
    xT_d = nc.dram_tensor("xT", [DIM, TOK], bf16, kind="ExternalInput")
    wqkvT_d = nc.dram_tensor("wqkvT", [DIM, 384], bf16, kind="ExternalInput")
    bias_d = nc.dram_tensor("bias", [128, 3], f32, kind="ExternalInput")
    wprojT_d = nc.dram_tensor("wprojT", [128, DIM], bf16, kind="ExternalInput")
    ident_d = nc.dram_tensor("ident", [128, 128], bf16, kind="ExternalInput")
    swap_d = nc.dram_tensor("swap", [128, 128], bf16, kind="ExternalInput")
    out_d = nc.dram_tensor("out", [TOK, DIM], bf16, kind="ExternalOutput")

    with TileContext(nc) as tc:
        with tc.tile_pool(name="sbuf", bufs=1) as sb, \
             tc.tile_pool(name="psum", bufs=1, space="PSUM") as ps:
            # constants / weights
            # weights/constants go out on the ACT engine's DMA queue so their
            # issue overlaps the x staging on the sync queue (ACT is idle in
            # the prologue).
            wqkv_t = sb.tile([128, NB_C, 384], bf16, tag="wqkv")
            _wsrc = wqkvT_d[:, :].rearrange("(ct p) r -> p ct r", p=128)
            for ct in range(0, NB_C, 2):
                nc.scalar.dma_start(wqkv_t[:, ct:ct + 2, :],
                                    _wsrc[:, ct:ct + 2, :])
            wproj_t = sb.tile([128, DIM], bf16, tag="wproj")
            nc.scalar.dma_start(wproj_t, wprojT_d[:, :])
            bias_t = sb.tile([128, 3], f32, tag="bias")
            nc.scalar.dma_start(bias_t, bias_d[:, :])
            ident_f = sb.tile([128, 128], bf16, tag="ident")
            nc.scalar.dma_start(ident_f, ident_d[:, :])
            swap_t = sb.tile([128, 128], bf16, tag="swap")
            nc.scalar.dma_start(swap_t, swap_d[:, :])
            ones_t = sb.tile([128, 1], bf16, tag="ones")
            nc.vector.memset(ones_t, 1.0)
            # preload the exp table set during the DMA lead-in
            warm_t = sb.tile([128, 1], f32, tag="warm")
            nc.scalar.activation(warm_t, ones_t, Exp)

            def alloc_batch_tiles():
                qT_t = sb.tile([128, N], bf16, tag="qT", bufs=2)
                kT_t = sb.tile([128, N], bf16, tag="kT", bufs=2)
                # v laid out [tok128, head, ktile, 128] with ones columns:
                # head A block cols = [v_A(64) | ones(64)], head B = [ones | v_B]
                v_t = sb.tile([128, 2, NB_J, 128], bf16, tag="v", bufs=2)
                nc.vector.tensor_copy(
                    v_t[:, 0, :, 64:128],
                    ones_t[:, None, :].broadcast_to([128, NB_J, 64]))
                nc.vector.tensor_copy(
                    v_t[:, 1, :, 0:64],
                    ones_t[:, None, :].broadcast_to([128, NB_J, 64]))
                return qT_t, kT_t, v_t

            def dma_xstage(b, tch, split=1):
                xst = sb.tile([128, NB_C, 512], bf16, tag="xst", bufs=4)
                t0 = b * N + tch * 512
                src = (xT_d[:, t0:t0 + 512]
                       .rearrange("(ct p) t -> p ct t", p=128))
                step = NB_C // split
                for c0 in range(0, NB_C, step):
                    nc.sync.dma_start(xst[:, c0:c0 + step, :],
                                      src[:, c0:c0 + step, :])
                return xst

            def qkv_items(tiles, tch, xst):
                # q/k/v projections for one 512-token chunk, sliced into
                # ~2-matmul filler items so no single slot of the attention
                # loop carries a bulky block that would delay the next S pair
                # (and thereby starve the exp engine).
                qT_t, kT_t, v_t = tiles
                state = {}
                items = []

                def mk(r, c0):
                    def f():
                        if c0 == 0:
                            state[r] = ps.tile([128, 512], f32, name="qp",
                                               tag="misc", bufs=2)
                        qp = state[r]
                        for ct in (c0, c0 + 1):
                            nc.tensor.matmul(
                                qp, wqkv_t[:, ct, r * 128:(r + 1) * 128],
                                xst[:, ct, :],
                                start=(ct == 0), stop=(ct == NB_C - 1))
                        if c0 == NB_C - 2:
                            if r == 0:
                                nc.vector.tensor_scalar_add(
                                    qT_t[:, tch * 512:(tch + 1) * 512], qp,
                                    bias_t[:, 0:1])
                            elif r == 1:
                                nc.vector.tensor_scalar_add(
                                    kT_t[:, tch * 512:(tch + 1) * 512], qp,
                                    bias_t[:, 1:2])
                            else:
                                vst = sb.tile([128, 512], bf16, name="vst",
                                              tag="vtst", bufs=2)
                                nc.vector.tensor_copy(vst, qp)
                                state["vst"] = vst
                    return f

                def mk_tr(s0):
                    def f():
                        vst = state["vst"]
                        for s in (s0, s0 + 1):
                            trp = ps.tile([128, 128], bf16, name="trp",
                                          tag="misc", bufs=2)
                            nc.tensor.transpose(
                                trp, vst[:, s * 128:(s + 1) * 128], ident_f)
                            j = tch * 4 + s
                            nc.vector.tensor_copy(v_t[:, 0, j, 0:64],
                                                  trp[:, 0:64])
                            nc.vector.tensor_copy(v_t[:, 1, j, 64:128],
                                                  trp[:, 64:128])
                    return f

                for r in (0, 1, 2):
                    for c0 in range(0, NB_C, 2):
                        items.append(mk(r, c0))
                items += [mk_tr(0), mk_tr(2)]
                return items

            def proj_mm(prev, idx, dma_eng=None):
                # one (ts, fc) output tile of the deferred projection
                ot_p, b_p, qc_p = prev
                ts, fc = divmod(idx, 2)
                pj = ps.tile([128, 512], f32, tag="misc", bufs=2)
                nc.tensor.matmul(
                    pj, ot_p[:, ts * 128:(ts + 1) * 128],
                    wproj_t[:, fc * 512:(fc + 1) * 512], start=True, stop=True)
                ost = sb.tile([128, 512], bf16, tag="ost", bufs=6)
                nc.vector.tensor_copy(ost, pj)
                row0 = b_p * N + qc_p * 512 + ts * 128
                (dma_eng or nc.sync).dma_start(
                    out_d[row0:row0 + 128, fc * 512:(fc + 1) * 512], ost)

            def finish_norm(pending):
                # swap Z rows onto O rows' lanes via a permutation matmul,
                # then reciprocal + normalization fused into the PSUM drain.
                z_p, oA_p, oB_p, b_p, qc_p = pending
                zsw = ps.tile([128, 512], f32, tag="misc", bufs=2)
                nc.tensor.matmul(zsw, swap_t, z_p, start=True, stop=True)
                r_t = sb.tile([128, 512], f32, tag="rt", bufs=2)
                nc.vector.reciprocal_approx_fast(out=r_t, in_=zsw)
                ot = sb.tile([128, 512], bf16, tag="ot", bufs=2)
                nc.vector.tensor_mul(ot[0:64, :], oA_p[0:64, :], r_t[0:64, :])
                nc.vector.tensor_mul(ot[64:128, :], oB_p[64:128, :],
                                     r_t[64:128, :])
                return (ot, b_p, qc_p)

            # ---- prologue: full QKV for batch 0 ----
            tiles = alloc_batch_tiles()
            xsts = [dma_xstage(0, t, split=s)
                    for t, s in zip(range(NB_TCH), (8, 8, 4, 4))]
            for tch in range(NB_TCH):
                for it in qkv_items(tiles, tch, xsts[tch]):
                    it()

            prev = None      # deferred projection: (ot, b, qc)
            pending = None   # deferred normalization: (z, oA, oB, b, qc)
            xst_q = dma_xstage(1, 0) if B > 1 else None
            for b in range(B):
                qT_t, kT_t, v_t = tiles
                if b + 1 < B:
                    next_tiles = alloc_batch_tiles()
                for qc in range(NB_QC):
                    # x chunk for this qc's qkv filler was prefetched one qc
                    # ago; issue the following one now.
                    xst_cur = xst_q
                    nb, nqc = (b, qc + 1) if qc + 1 < NB_QC else (b + 1, 0)
                    xst_q = (dma_xstage(nb + 1, nqc)
                             if nb + 1 < B else None)
                    fillers = (qkv_items(next_tiles, qc, xst_cur)
                               if b + 1 < B else [])
                    q_sl = slice(qc * 512, (qc + 1) * 512)
                    oA = oB = None  # allocated lazily at first PV use: the
                    # pool's WAR-wait (vs the previous qc's z-copies) must not
                    # sit ahead of this qc's S matmuls in the PE queue.
                    # j-loop at key-tile-PAIR granularity: grouping the
                    # 64-row QK MMs and 128-row PV MMs reduces row-shape
                    # transitions so LDWEIGHTS hides under matmuls. Previous
                    # qc's proj and next batch's qkv chunks fill PE while ACT
                    # runs exp. The S schedule is staggered (1 tile in the
                    # first and last slots, 2 in between) so the first S of a
                    # qc only needs the st slot of j14, which ACT has already
                    # drained -- no boundary wait on the trailing exp.
                    s_at = {0: (0,), 8: (15,)}
                    for mm_ in range(1, 8):
                        s_at[mm_] = (2 * mm_ - 1, 2 * mm_)
                    pv_at = {9: (14, 15)}
                    for mm_ in range(2, 9):
                        pv_at[mm_] = (2 * mm_ - 4, 2 * mm_ - 3)
                    e_pend = [None] * NB_J
                    for m in range(10):
                        for j in s_at.get(m, ()):
                            k_sl = slice(j * 128, (j + 1) * 128)
                            st = ps.tile([128, 1024], f32, tag="st", bufs=2)
                            nc.tensor.matmul(
                                st[:, 0:512], kT_t[0:64, k_sl],
                                qT_t[0:64, q_sl], start=True, stop=True)
                            nc.tensor.matmul(
                                st[:, 512:1024], kT_t[64:128, k_sl],
                                qT_t[64:128, q_sl], start=True, stop=True,
                                tile_position=(64, 0))
                            e_t = sb.tile([128, 1024], bf16, tag="e", bufs=8)
                            nc.scalar.activation(e_t, st, Exp)
                            e_pend[j] = e_t
                        if m == 0 and pending is not None:
                            prev = finish_norm(pending)
                            pending = None
                        if prev is not None and 2 <= m <= 9:
                            proj_mm(prev, m - 2)
                        take = 2 if m < 4 else 1
                        while take and fillers:
                            fillers.pop(0)()
                            take -= 1
                        for j in pv_at.get(m, ()):
                            if oA is None:
                                oA = ps.tile([128, 512], f32, tag="oA",
                                             bufs=1)
                                oB = ps.tile([128, 512], f32, tag="oB",
                                             bufs=1)
                            e_p = e_pend[j]
                            nc.tensor.matmul(
                                oA, v_t[:, 0, j, :], e_p[:, 0:512],
                                start=(j == 0), stop=(j == NB_J - 1))
                            nc.tensor.matmul(
                                oB, v_t[:, 1, j, :], e_p[:, 512:1024],
                                start=(j == 0), stop=(j == NB_J - 1))
                    # stage Z into SBUF now; the two halves go out on
                    # different engines (ACT has drained its queue by the qc
                    # boundary) so the chain clears the PE's WAR-wait sooner.
                    z_st = sb.tile([128, 512], bf16, tag="zst", bufs=2)
                    nc.scalar.copy(z_st[64:128, :], oA[64:128, :])
                    nc.vector.tensor_copy(z_st[0:64, :], oB[0:64, :])
                    pending = (z_st, oA, oB, b, qc)
                if b + 1 < B:
                    tiles = next_tiles
            # tail: last qc's normalization + projection; alternate the
            # output DMAs across both HWDGE queues (ACT is idle by now) to
            # shorten the final drain.
            prev = finish_norm(pending)
            for idx in range(8):
                proj_mm(prev, idx, dma_eng=(nc.scalar if idx % 2 else None))

    nc.compile()
    _cache["nc"] = nc
    return nc


def _ensure_ntff_hook():
    """Register the axon NTFF profile hook (antenv.axon_hooks) if absent.

    The agent image's antenv stub lacks axon_hooks, so trn_boot's hook
    registration silently degrades; recreate it here via the same ctypes
    recipe so run_bass_kernel_spmd(trace=True) can capture HW profiles.
    """
    import sys
    import types
    import ctypes
    import contextlib

    try:
        from antenv.axon_hooks import get_axon_ntff_profile_hook
        if get_axon_ntff_profile_hook() is not None:
            return
    except ImportError:
        mod = types.ModuleType("antenv.axon_hooks")
        mod._hook = None
        mod.get_axon_ntff_profile_hook = lambda: mod._hook

        def _set(h):
            mod._hook = h
        mod.set_axon_ntff_profile_hook = _set
        sys.modules["antenv.axon_hooks"] = mod
        import antenv
        antenv.axon_hooks = mod

    so_path = "/opt/axon/libaxon_pjrt.so"
    if not os.path.exists(so_path):
        return
    lib = ctypes.CDLL(so_path)
    if not hasattr(lib, "axon_start_nrt_profile"):
        return
    lib.axon_start_nrt_profile.argtypes = [
        ctypes.POINTER(ctypes.c_int64), ctypes.c_size_t]
    lib.axon_start_nrt_profile.restype = ctypes.c_int64
    lib.axon_stop_nrt_profile.argtypes = [ctypes.c_char_p]
    lib.axon_stop_nrt_profile.restype = ctypes.c_int64

    @contextlib.contextmanager
    def _hook(output_dir, device_ids):
        # the .so's GLOBAL_CLIENT is only set once something executes
        import jax
        jax.block_until_ready(
            jax.jit(lambda a: a + 1)(jax.numpy.zeros((8,), jax.numpy.float32)))
        if device_ids:
            ids = (ctypes.c_int64 * len(device_ids))(*device_ids)
            rc = lib.axon_start_nrt_profile(ids, len(device_ids))
        else:
            rc = lib.axon_start_nrt_profile(None, 0)
        if rc != 0:
            raise RuntimeError(f"axon_start_nrt_profile rc={rc}")
        try:
            yield
        finally:
            n = lib.axon_stop_nrt_profile(str(output_dir).encode())
            print(f"profile: {n} file(s) written to {output_dir}")

    from antenv.axon_hooks import set_axon_ntff_profile_hook
    set_axon_ntff_profile_hook(_hook)


def kernel(x, w_qkv, b_qkv, w_proj, b_proj):
    import ml_dtypes
    from concourse.bass_utils import run_bass_kernel_spmd

    bf16 = ml_dtypes.bfloat16
    nc = _build()
    x = np.asarray(x, dtype=np.float32)
    w_qkv = np.asarray(w_qkv, dtype=np.float32)
    b_qkv = np.asarray(b_qkv, dtype=np.float32)
    w_proj = np.asarray(w_proj, dtype=np.float32)
    b_proj = np.asarray(b_proj, dtype=np.float32)

    xT = np.ascontiguousarray(x.reshape(TOK, DIM).T).astype(bf16)
    ident = np.eye(128, dtype=np.float32).astype(bf16)
    # permutation that swaps partition halves: out = swap.T @ z
    swap = np.zeros((128, 128), dtype=np.float32)
    swap[np.arange(64) + 64, np.arange(64)] = 1.0
    swap[np.arange(64), np.arange(64) + 64] = 1.0
    swap = swap.astype(bf16)

    in_maps = []
    for c in range(N_CORES):
        sl = slice(HEAD_DIM * 2 * c, HEAD_DIM * 2 * c + 128)
        wq = w_qkv[0 * DIM:1 * DIM][sl] * SCALE
        wk = w_qkv[1 * DIM:2 * DIM][sl]
        wv = w_qkv[2 * DIM:3 * DIM][sl]
        wqkvT = np.ascontiguousarray(
            np.concatenate([wq, wk, wv], 0).T).astype(bf16)
        bq = b_qkv[0 * DIM:1 * DIM][sl] * SCALE
        bk = b_qkv[1 * DIM:2 * DIM][sl]
        bias = np.ascontiguousarray(
            np.stack([bq, bk, np.zeros_like(bq)], 1))
        wprojT = np.ascontiguousarray(w_proj[:, sl].T).astype(bf16)
        in_maps.append({"xT": xT, "wqkvT": wqkvT, "bias": bias,
                        "wprojT": wprojT, "ident": ident, "swap": swap})

    trace = os.environ.get("BASS_KERNEL_TRACE", "0") == "1"
    if trace:
        _ensure_ntff_hook()
    res = run_bass_kernel_spmd(nc, in_maps, list(range(N_CORES)), trace=trace)
    if trace:
        _cache["last_exec_time_ns"] = res.exec_time_ns
        _cache["last_mean_exec_time_ns"] = res.mean_exec_time_ns

    out = np.zeros((TOK, DIM), dtype=np.float64)
    for c in range(N_CORES):
        out += res.results[c]["out"].astype(np.float64)
    # v-bias contributes a constant (softmax weights sum to 1): fold into
    # the projection bias here instead of adding it on-device.
    out += b_proj + b_qkv[2 * DIM:3 * DIM] @ w_proj.T
    return out.reshape(B, N, DIM).astype(np.float32)


# revision 41
# speedup vs baseline: 1.1999x; 1.0088x over previous
"""Multi-head attention (nn_Attention) for 8 Trainium2 NeuronCores.

Sharding: tensor-parallel over heads (2 heads per core). Each core computes
qkv projection for its head slice from the full input, full attention for its
2 heads, and a partial output projection; partials are summed on the host.

Layout strategy (per core):
  - qkv^T = W_slice @ x^T computed with contraction (c=1024) on the partition
    dim; produces q^T/k^T [128=2*64 head dims, tokens] directly in the
    orientation the S^T matmuls need.
  - S^T tiles [128 keys, 512 queries x 2 heads] via row-tiled matmul pairs
    (head A on array rows 0:63, head B on 64:127) which execute CONCURRENTLY
    on the PE's row groups.
  - softmax without max-subtraction (|S| < 7 for these inputs): exp on ACT
    (PSUM -> SBUF bf16), then O^T = (E^T [v|ones]) with the ones columns
    producing the softmax normalizer Z on the opposite 64 partitions.
  - Z rows are moved onto the O rows' partitions with a swap-halves
    permutation matmul, reciprocal via the fast custom DVE op, and the
    normalization fused into the PSUM->SBUF copy (tensor_mul). The chain is
    deferred into the next qc's first slot so it never blocks the S/exp
    pipeline at qc boundaries.
  - proj: out_partial[tokens, feat] = O^T_cat.T @ w_projT_slice, summed on
    host across cores. v-bias is folded into b_proj on the host (softmax
    weights sum to 1, so the v bias adds a constant to O).
All matmul operands are bf16 (fp32 streams at ~2 cycles/row on HW, bf16 at
1); intermediates accumulate in fp32 PSUM. Output partials ship as bf16.
"""

import os
import numpy as np

N_CORES = 8
DIM = 1024
N_HEADS = 16
HEAD_DIM = 64
SCALE = HEAD_DIM ** -0.5
B, N = 4, 2048
TOK = B * N  # 8192
NB_C = DIM // 128   # 8 contraction tiles for qkv
NB_J = N // 128     # 16 key tiles per batch
NB_QC = N // 512    # 4 query chunks per batch
NB_TCH = N // 512   # 4 token chunks per batch (qkv)

_cache = {}


def _build():
    if "nc" in _cache:
        return _cache["nc"]
    import concourse.bacc as bacc
    import concourse.mybir as mybir
    from concourse.tile import TileContext

    f32 = mybir.dt.float32
    bf16 = mybir.dt.bfloat16
    Exp = mybir.ActivationFunctionType.Exp

    nc = bacc.Bacc(None, target_bir_lowering=False)
    xT_d = nc.dram_tensor("xT", [DIM, TOK], bf16, kind="ExternalInput")
    wqkvT_d = nc.dram_tensor("wqkvT", [DIM, 384], bf16, kind="ExternalInput")
    bias_d = nc.dram_tensor("bias", [128, 3], f32, kind="ExternalInput")
    wprojT_d = nc.dram_tensor("wprojT", [128, DIM], bf16, kind="ExternalInput")
    ident_d = nc.dram_tensor("ident", [128, 128], bf16, kind="ExternalInput")
    swap_d = nc.dram_tensor("swap", [128, 128], bf16, kind="ExternalInput")
    out_d = nc.dram_tensor("out", [TOK, DIM], bf16, kind="ExternalOutput")

    with TileContext(nc) as tc:
        with tc.tile_pool(name="sbuf", bufs=1) as sb, \
             tc.tile_pool(name="psum", bufs=1, space="PSUM") as ps:
            # constants / weights
            # weights/constants go out on the ACT engine's DMA queue so their
            # issue overlaps the x staging on the sync queue (ACT is idle in
            # the prologue).
            wqkv_t = sb.tile([128, NB_C, 384], bf16, tag="wqkv")
            _wsrc = wqkvT_d[:, :].rearrange("(ct p) r -> p ct r", p=128)
            for ct in range(0, NB_C, 2):
                nc.scalar.dma_start(wqkv_t[:, ct:ct + 2, :],
                                    _wsrc[:, ct:ct + 2, :])
            wproj_t = sb.tile([128, DIM], bf16, tag="wproj")
            nc.scalar.dma_start(wproj_t, wprojT_d[:, :])
            bias_t = sb.tile([128, 3], f32, tag="bias")
            nc.scalar.dma_start(bias_t, bias_d[:, :])
            ident_f = sb.tile([128, 128], bf16, tag="ident")
            nc.scalar.dma_start(ident_f, ident_d[:, :])
            swap_t = sb.tile([128, 128], bf16, tag="swap")
            nc.scalar.dma_start(swap_t, swap_d[:, :])
            ones_t = sb.tile([128, 1], bf16, tag="ones")
            nc.vector.memset(ones_t, 1.0)
            # preload the exp table set during the DMA lead-in
            warm_t = sb.tile([128, 1], f32, tag="warm")
            nc.scalar.activation(warm_t, ones_t, Exp)

            def alloc_batch_tiles():
                qT_t = sb.tile([128, N], bf16, tag="qT", bufs=2)
                kT_t = sb.tile([128, N], bf16, tag="kT", bufs=2)
                # v laid out [tok128, head, ktile, 128] with ones columns:
                # head A block cols = [v_A(64) | ones(64)], head B = [ones | v_B]
                v_t = sb.tile([128, 2, NB_J, 128], bf16, tag="v", bufs=2)
                nc.vector.tensor_copy(
                    v_t[:, 0, :, 64:128],
                    ones_t[:, None, :].broadcast_to([128, NB_J, 64]))
                nc.vector.tensor_copy(
                    v_t[:, 1, :, 0:64],
                    ones_t[:, None, :].broadcast_to([128, NB_J, 64]))
                return qT_t, kT_t, v_t

            def dma_xstage(b, tch, split=1):
                xst = sb.tile([128, NB_C, 512], bf16, tag="xst", bufs=4)
                t0 = b * N + tch * 512
                src = (xT_d[:, t0:t0 + 512]
                       .rearrange("(ct p) t -> p ct t", p=128))
                step = NB_C // split
                for c0 in range(0, NB_C, step):
                    nc.sync.dma_start(xst[:, c0:c0 + step, :],
                                      src[:, c0:c0 + step, :])
                return xst

            def qkv_items(tiles, tch, xst):
                # q/k/v projections for one 512-token chunk, sliced into
                # ~2-matmul filler items so no single slot of the attention
                # loop carries a bulky block that would delay the next S pair
                # (and thereby starve the exp engine).
                qT_t, kT_t, v_t = tiles
                state = {}
                items = []

                def mk(r, c0):
                    def f():
                        if c0 == 0:
                            state[r] = ps.tile([128, 512], f32, name="qp",
                                               tag="misc", bufs=2)
                        qp = state[r]
                        for ct in (c0, c0 + 1):
                            nc.tensor.matmul(
                                qp, wqkv_t[:, ct, r * 128:(r + 1) * 128],
                                xst[:, ct, :],
                                start=(ct == 0), stop=(ct == NB_C - 1))
                        if c0 == NB_C - 2:
                            if r == 0:
                                nc.vector.tensor_scalar_add(
                                    qT_t[:, tch * 512:(tch + 1) * 512], qp,
                                    bias_t[:, 0:1])
                            elif r == 1:
                                nc.vector.tensor_scalar_add(
                                    kT_t[:, tch * 512:(tch + 1) * 512], qp,
                                    bias_t[:, 1:2])
                            else:
                                vst = sb.tile([128, 512], bf16, name="vst",
                                              tag="vtst", bufs=2)
                                nc.vector.tensor_copy(vst, qp)
                                state["vst"] = vst
                    return f

                def mk_tr(s0):
                    def f():
                        vst = state["vst"]
                        for s in (s0, s0 + 1):
                            trp = ps.tile([128, 128], bf16, name="trp",
                                          tag="misc", bufs=2)
                            nc.tensor.transpose(
                                trp, vst[:, s * 128:(s + 1) * 128], ident_f)
                            j = tch * 4 + s
                            nc.vector.tensor_copy(v_t[:, 0, j, 0:64],
                                                  trp[:, 0:64])
                            nc.vector.tensor_copy(v_t[:, 1, j, 64:128],
                                                  trp[:, 64:128])
                    return f

                for r in (0, 1, 2):
                    for c0 in range(0, NB_C, 2):
                        items.append(mk(r, c0))
                items += [mk_tr(0), mk_tr(2)]
                return items

            def proj_mm(prev, idx):
                # one (ts, fc) output tile of the deferred projection
                ot_p, b_p, qc_p = prev
                ts, fc = divmod(idx, 2)
                pj = ps.tile([128, 512], f32, tag="misc", bufs=2)
                nc.tensor.matmul(
                    pj, ot_p[:, ts * 128:(ts + 1) * 128],
                    wproj_t[:, fc * 512:(fc + 1) * 512], start=True, stop=True)
                ost = sb.tile([128, 512], bf16, tag="ost", bufs=6)
                nc.vector.tensor_copy(ost, pj)
                row0 = b_p * N + qc_p * 512 + ts * 128
                nc.sync.dma_start(
                    out_d[row0:row0 + 128, fc * 512:(fc + 1) * 512], ost)

            def finish_norm(pending):
                # swap Z rows onto O rows' lanes via a permutation matmul,
                # then reciprocal + normalization fused into the PSUM drain.
                z_p, oA_p, oB_p, b_p, qc_p = pending
                zsw = ps.tile([128, 512], f32, tag="misc", bufs=2)
                nc.tensor.matmul(zsw, swap_t, z_p, start=True, stop=True)
                r_t = sb.tile([128, 512], f32, tag="rt", bufs=2)
                nc.vector.reciprocal_approx_fast(out=r_t, in_=zsw)
                ot = sb.tile([128, 512], bf16, tag="ot", bufs=2)
                nc.vector.tensor_mul(ot[0:64, :], oA_p[0:64, :], r_t[0:64, :])
                nc.vector.tensor_mul(ot[64:128, :], oB_p[64:128, :],
                                     r_t[64:128, :])
                return (ot, b_p, qc_p)

            # ---- prologue: full QKV for batch 0 ----
            tiles = alloc_batch_tiles()
            xsts = [dma_xstage(0, t, split=s)
                    for t, s in zip(range(NB_TCH), (8, 8, 4, 4))]
            for tch in range(NB_TCH):
                for it in qkv_items(tiles, tch, xsts[tch]):
                    it()

            prev = None      # deferred projection: (ot, b, qc)
            pending = None   # deferred normalization: (z, oA, oB, b, qc)
            xst_q = dma_xstage(1, 0) if B > 1 else None
            for b in range(B):
                qT_t, kT_t, v_t = tiles
                if b + 1 < B:
                    next_tiles = alloc_batch_tiles()
                for qc in range(NB_QC):
                    # x chunk for this qc's qkv filler was prefetched one qc
                    # ago; issue the following one now.
                    xst_cur = xst_q
                    nb, nqc = (b, qc + 1) if qc + 1 < NB_QC else (b + 1, 0)
                    xst_q = (dma_xstage(nb + 1, nqc)
                             if nb + 1 < B else None)
                    fillers = (qkv_items(next_tiles, qc, xst_cur)
                               if b + 1 < B else [])
                    q_sl = slice(qc * 512, (qc + 1) * 512)
                    oA = oB = None  # allocated lazily at first PV use: the
                    # pool's WAR-wait (vs the previous qc's z-copies) must not
                    # sit ahead of this qc's S matmuls in the PE queue.
                    # j-loop at key-tile-PAIR granularity: grouping the
                    # 64-row QK MMs and 128-row PV MMs reduces row-shape
                    # transitions so LDWEIGHTS hides under matmuls. Previous
                    # qc's proj and next batch's qkv chunks fill PE while ACT
                    # runs exp. The S schedule is staggered (1 tile in the
                    # first and last slots, 2 in between) so the first S of a
                    # qc only needs the st slot of j14, which ACT has already
                    # drained -- no boundary wait on the trailing exp.
                    s_at = {0: (0,), 8: (15,)}
                    for mm_ in range(1, 8):
                        s_at[mm_] = (2 * mm_ - 1, 2 * mm_)
                    pv_at = {9: (14, 15)}
                    for mm_ in range(2, 9):
                        pv_at[mm_] = (2 * mm_ - 4, 2 * mm_ - 3)
                    e_pend = [None] * NB_J
                    for m in range(10):
                        for j in s_at.get(m, ()):
                            k_sl = slice(j * 128, (j + 1) * 128)
                            st = ps.tile([128, 1024], f32, tag="st", bufs=2)
                            nc.tensor.matmul(
                                st[:, 0:512], kT_t[0:64, k_sl],
                                qT_t[0:64, q_sl], start=True, stop=True)
                            nc.tensor.matmul(
                                st[:, 512:1024], kT_t[64:128, k_sl],
                                qT_t[64:128, q_sl], start=True, stop=True,
                                tile_position=(64, 0))
                            e_t = sb.tile([128, 1024], bf16, tag="e", bufs=8)
                            nc.scalar.activation(e_t, st, Exp)
                            e_pend[j] = e_t
                        if m == 0 and pending is not None:
                            prev = finish_norm(pending)
                            pending = None
                        if prev is not None and 1 <= m <= 8:
                            proj_mm(prev, m - 1)
                        take = 2 if m < 4 else 1
                        while take and fillers:
                            fillers.pop(0)()
                            take -= 1
                        for j in pv_at.get(m, ()):
                            if oA is None:
                                oA = ps.tile([128, 512], f32, tag="oA",
                                             bufs=1)
                                oB = ps.tile([128, 512], f32, tag="oB",
                                             bufs=1)
                            e_p = e_pend[j]
                            nc.tensor.matmul(
                                oA, v_t[:, 0, j, :], e_p[:, 0:512],
                                start=(j == 0), stop=(j == NB_J - 1))
                            nc.tensor.matmul(
                                oB, v_t[:, 1, j, :], e_p[:, 512:1024],
                                start=(j == 0), stop=(j == NB_J - 1))
                    # stage Z into SBUF now; the two halves go out on
                    # different engines (ACT has drained its queue by the qc
                    # boundary) so the chain clears the PE's WAR-wait sooner.
                    z_st = sb.tile([128, 512], bf16, tag="zst", bufs=2)
                    nc.scalar.copy(z_st[64:128, :], oA[64:128, :])
                    nc.vector.tensor_copy(z_st[0:64, :], oB[0:64, :])
                    pending = (z_st, oA, oB, b, qc)
                if b + 1 < B:
                    tiles = next_tiles
            # tail: last qc's normalization + projection
            prev = finish_norm(pending)
            for idx in range(8):
                proj_mm(prev, idx)

    nc.compile()
    _cache["nc"] = nc
    return nc


def _ensure_ntff_hook():
    """Register the axon NTFF profile hook (antenv.axon_hooks) if absent.

    The agent image's antenv stub lacks axon_hooks, so trn_boot's hook
    registration silently degrades; recreate it here via the same ctypes
    recipe so run_bass_kernel_spmd(trace=True) can capture HW profiles.
    """
    import sys
    import types
    import ctypes
    import contextlib

    try:
        from antenv.axon_hooks import get_axon_ntff_profile_hook
        if get_axon_ntff_profile_hook() is not None:
            return
    except ImportError:
        mod = types.ModuleType("antenv.axon_hooks")
        mod._hook = None
        mod.get_axon_ntff_profile_hook = lambda: mod._hook

        def _set(h):
            mod._hook = h
        mod.set_axon_ntff_profile_hook = _set
        sys.modules["antenv.axon_hooks"] = mod
        import antenv
        antenv.axon_hooks = mod

    so_path = "/opt/axon/libaxon_pjrt.so"
    if not os.path.exists(so_path):
        return
    lib = ctypes.CDLL(so_path)
    if not hasattr(lib, "axon_start_nrt_profile"):
        return
    lib.axon_start_nrt_profile.argtypes = [
        ctypes.POINTER(ctypes.c_int64), ctypes.c_size_t]
    lib.axon_start_nrt_profile.restype = ctypes.c_int64
    lib.axon_stop_nrt_profile.argtypes = [ctypes.c_char_p]
    lib.axon_stop_nrt_profile.restype = ctypes.c_int64

    @contextlib.contextmanager
    def _hook(output_dir, device_ids):
        # the .so's GLOBAL_CLIENT is only set once something executes
        import jax
        jax.block_until_ready(
            jax.jit(lambda a: a + 1)(jax.numpy.zeros((8,), jax.numpy.float32)))
        if device_ids:
            ids = (ctypes.c_int64 * len(device_ids))(*device_ids)
            rc = lib.axon_start_nrt_profile(ids, len(device_ids))
        else:
            rc = lib.axon_start_nrt_profile(None, 0)
        if rc != 0:
            raise RuntimeError(f"axon_start_nrt_profile rc={rc}")
        try:
            yield
        finally:
            n = lib.axon_stop_nrt_profile(str(output_dir).encode())
            print(f"profile: {n} file(s) written to {output_dir}")

    from antenv.axon_hooks import set_axon_ntff_profile_hook
    set_axon_ntff_profile_hook(_hook)


def kernel(x, w_qkv, b_qkv, w_proj, b_proj):
    import ml_dtypes
    from concourse.bass_utils import run_bass_kernel_spmd

    bf16 = ml_dtypes.bfloat16
    nc = _build()
    x = np.asarray(x, dtype=np.float32)
    w_qkv = np.asarray(w_qkv, dtype=np.float32)
    b_qkv = np.asarray(b_qkv, dtype=np.float32)
    w_proj = np.asarray(w_proj, dtype=np.float32)
    b_proj = np.asarray(b_proj, dtype=np.float32)

    xT = np.ascontiguousarray(x.reshape(TOK, DIM).T).astype(bf16)
    ident = np.eye(128, dtype=np.float32).astype(bf16)
    # permutation that swaps partition halves: out = swap.T @ z
    swap = np.zeros((128, 128), dtype=np.float32)
    swap[np.arange(64) + 64, np.arange(64)] = 1.0
    swap[np.arange(64), np.arange(64) + 64] = 1.0
    swap = swap.astype(bf16)

    in_maps = []
    for c in range(N_CORES):
        sl = slice(HEAD_DIM * 2 * c, HEAD_DIM * 2 * c + 128)
        wq = w_qkv[0 * DIM:1 * DIM][sl] * SCALE
        wk = w_qkv[1 * DIM:2 * DIM][sl]
        wv = w_qkv[2 * DIM:3 * DIM][sl]
        wqkvT = np.ascontiguousarray(
            np.concatenate([wq, wk, wv], 0).T).astype(bf16)
        bq = b_qkv[0 * DIM:1 * DIM][sl] * SCALE
        bk = b_qkv[1 * DIM:2 * DIM][sl]
        bias = np.ascontiguousarray(
            np.stack([bq, bk, np.zeros_like(bq)], 1))
        wprojT = np.ascontiguousarray(w_proj[:, sl].T).astype(bf16)
        in_maps.append({"xT": xT, "wqkvT": wqkvT, "bias": bias,
                        "wprojT": wprojT, "ident": ident, "swap": swap})

    trace = os.environ.get("BASS_KERNEL_TRACE", "0") == "1"
    if trace:
        _ensure_ntff_hook()
    res = run_bass_kernel_spmd(nc, in_maps, list(range(N_CORES)), trace=trace)
    if trace:
        _cache["last_exec_time_ns"] = res.exec_time_ns
        _cache["last_mean_exec_time_ns"] = res.mean_exec_time_ns

    out = np.zeros((TOK, DIM), dtype=np.float64)
    for c in range(N_CORES):
        out += res.results[c]["out"].astype(np.float64)
    # v-bias contributes a constant (softmax weights sum to 1): fold into
    # the projection bias here instead of adding it on-device.
    out += b_proj + b_qkv[2 * DIM:3 * DIM] @ w_proj.T
    return out.reshape(B, N, DIM).astype(np.float32)
